# revision 4
# baseline (speedup 1.0000x reference)
"""Single-head GAT (DGL GATConv) forward on 8 Trainium2 NeuronCores — v3.

Cost model (measured): ~50-100us fixed per instruction regardless of size;
dma_gather moves <=1024 rows per instruction (SWDGE ucode cap);
dma_scatter_add races on duplicate indices on HW (unusable here).

Design: minimize instruction count.
  - Node rows [h+bias (256 bf16) | el f32 (bitcast 2) | er f32 (bitcast 2) |
    0 pad] = 384 bf16 (768 B), AllGathered to hfull [10240, 384].
  - Edges grouped per dst into R=16-slot groups (per-dst padding to 16);
    10 windows/core of 128 dst; per-window stream padded to 6144 slots
    (384 groups).  Stream gathered by src via dma_gather (1024 rows/op).
  - Per-edge weight w = exp(lrelu(el_src + er_dst)) on DVE/ACT in batches
    of 2 windows; rows weighted in place; R-reduce -> 384 group rows per
    window [Sum w*h | Sum w | junk] f32.
  - Group->dst scatter via small Sel matmul: 3 chunks of 128 groups per
    window, fp32 PE, psum [128, 257].
  - Epilogue: out = num/denom (one reciprocal + one multiply + one DMA).

kernel(**inputs) takes full unsharded inputs, returns [10000, 256] fp32.
"""

import numpy as np
import ml_dtypes

N = 10000
E = 320000
D = 256
NPAD = 10240
NCORES = 8
SH = NPAD // NCORES          # 1280 nodes per core
WINN = 128                   # dst nodes per window
NW = SH // WINN              # 10 windows per core
R = 16                       # edge slots per reduce-group
GW = 384                     # groups per window (3 chunks of 128)
SLOTS_W = GW * R             # 6144 edge slots per window
CHW = SLOTS_W // 128         # 48 gather-chunks per window
NBW = 2                      # windows per DVE batch
NBATCH = NW // NBW           # 5 batches
GB = 1024                    # rows per dma_gather
NG_W = SLOTS_W // GB         # 6 gathers per window
DA = 384                     # bf16 elements per node row (768 B)
UW = 257                     # used columns of reduced rows
RW = 320                     # reduced row width f32 (1280 B)
NEG_SLOPE = 0.2
REPEAT = 1
DEBUG_RED = False            # add a DRAM dump of the reduced group rows
ABLATE_GATHER = False        # timing: memset instead of dma_gather
ABLATE_MM = False            # timing: memset ustage instead of sel matmuls
ABLATE_DVE = False           # timing: memset red instead of weight chain
ABLATE_COLL = False          # timing: skip AllGather
ABLATE_P1 = False            # timing: memset hbig instead of h matmuls
NQ = 1                       # SWDGE queues to round-robin gathers over

_BF16 = ml_dtypes.bfloat16

_prog_cache = {}


def _prep_inputs(feats, W, attn_l, attn_r, bias, src, dst, ncores=NCORES):
    """Host-side sharding/index prep. Returns (in_maps, newid)."""
    import heapq

    sh = NPAD // ncores
    nw_g = NPAD // WINN                       # 80 windows globally

    feats_pad = np.zeros((NPAD, D), np.float32)
    feats_pad[:N] = feats
    val = (W @ attn_l).astype(np.float32)
    var = (W @ attn_r).astype(np.float32)
    w_ext = np.concatenate(
        [W.astype(np.float32), val[:, None], var[:, None]], axis=1)  # [256,258]

    src = src.astype(np.int64)
    dst = dst.astype(np.int64)
    deg = np.bincount(dst, minlength=NPAD).astype(np.int64)
    gcount = -(-deg // R)                      # groups per node

    # heap-pack nodes into windows of 128, balancing group counts
    norder = np.argsort(-gcount, kind="stable")
    bin_groups = np.zeros(nw_g, np.int64)
    bin_count = np.zeros(nw_g, np.int64)
    newid = np.empty(NPAD, np.int64)
    heap = [(0, b) for b in range(nw_g)]
    heapq.heapify(heap)
    for n in norder:
        while True:
            _, b = heapq.heappop(heap)
            if bin_count[b] < WINN:
                break
        newid[n] = b * WINN + bin_count[b]
        bin_count[b] += 1
        bin_groups[b] += gcount[n]
        if bin_count[b] < WINN:
            heapq.heappush(heap, (bin_groups[b], b))
    assert bin_groups.max() <= GW, (bin_groups.max(), GW)
    inv = np.empty(NPAD, np.int64)
    inv[newid] = np.arange(NPAD)
    feats_pad = feats_pad[inv]                 # row j = new id j

    # one fake node id (h=0, el=0; er forced to -1e30 on device)
    fakes = newid[N:NPAD]
    assert len(fakes) > 0
    fid = int(fakes[0])
    is_fake = np.zeros(NPAD, bool)
    is_fake[fakes] = True

    n_src = newid[src]
    n_dst = newid[dst]
    order = np.argsort(n_dst, kind="stable")
    s_src = n_src[order]
    s_dst = n_dst[order]
    starts = np.searchsorted(s_dst, np.arange(NPAD + 1))

    # per-window edge stream (6144 slots) + group table (384 groups)
    gath = np.full((nw_g, SLOTS_W), fid, np.int64)   # src gather idx
    gslot = np.full((nw_g, GW), -1.0, np.float32)    # group -> dst slot
    gdst = np.full((nw_g, GW), fid, np.int64)        # group -> dst node id
    for w in range(nw_g):
        pos = 0
        gi = 0
        for s in range(WINN):
            node = w * WINN + s
            a, b = starts[node], starts[node + 1]
            k = b - a
            if k == 0:
                continue
            ng = -(-k // R)
            gath[w, pos:pos + k] = s_src[a:b]
            gslot[w, gi:gi + ng] = s
            gdst[w, gi:gi + ng] = node
            pos += ng * R
            gi += ng
        assert gi <= GW and pos <= SLOTS_W

    # stream slot i of window w -> token (p = i%128, chunk c = i//128);
    # group of token = (p, c//R); member j = c%R.
    # Slot numbering inside window must place group g's members at
    # chunks [ (g//128)*R , +R ) on partition g%128.  Build mapping:
    # linear group gi -> (p=gi%128, q=gi//128); member j -> stream pos
    # p + 128*(q*R + j).
    lin = np.arange(SLOTS_W)
    gi_of = (lin // R)                       # linear group id 0..383
    j_of = lin % R
    p_of = gi_of % 128
    q_of = gi_of // 128
    tok = p_of + 128 * (q_of * R + j_of)     # token position in window
    # gather index vector in token order: gtok[w, tok] = gath[w, lin]
    gtok = np.empty_like(gath)
    gtok[:, tok] = gath

    def wrap_idx(flat):
        # [.., n] -> [128, .., n//16] int16 (i -> partition i%16, col i//16)
        nn = flat.shape[-1]
        w16 = flat.reshape(*flat.shape[:-1], nn // 16, 16)
        w16 = np.moveaxis(w16, -1, 0)                  # [16, .., n//16]
        rep = np.tile(w16, (8,) + (1,) * (w16.ndim - 1))
        return np.ascontiguousarray(rep.astype(np.int16))

    iota_row = np.ascontiguousarray(
        np.broadcast_to(np.arange(128, dtype=np.float32), (128, 128)))

    in_maps = []
    for c in range(ncores):
        wlo, whi = c * NW, c * NW + NW
        featsT = np.ascontiguousarray(feats_pad[c * sh:(c + 1) * sh].T)
        # gather idx: per core stream = concat of its windows' token streams
        stream = gtok[wlo:whi].reshape(-1)             # [61440]
        gidx = wrap_idx(stream.reshape(NW * NG_W, GB)) # [128, 60, 64]
        # er gather idx: per batch 768 groups (2 windows)
        gd = gdst[wlo:whi].reshape(NBATCH, NBW * GW)   # [5, 768]
        eidx = wrap_idx(gd)                            # [128, 5, 48]
        # slotf: [128, 30, 128]: chunk 3w+k partition p -> slot of group
        sl = gslot[wlo:whi].reshape(NW * 3, 128).T     # [128, 30]
        slotf = np.ascontiguousarray(
            np.broadcast_to(sl[:, :, None], (128, NW * 3, 1)).astype(
                np.float32))
        # eradd2 [128, 10, 2]: fake nodes get el = -1e30 so pad slots
        # (which gather the fake row by src but take er from their REAL
        # group dst) still produce w = exp(lrelu(-1e30 + er)) = 0.
        lid = (np.arange(NW)[None, :] * 128
               + np.arange(128)[:, None])              # [128, 10] local id
        fk = is_fake[c * sh + lid]
        eradd2 = np.zeros((128, NW, 2), np.float32)
        eradd2[:, :, 0] = np.where(fk, -1e30, 0.0)
        in_maps.append({
            "featsT": featsT,
            "Wext": w_ext,
            "bias_in": np.ascontiguousarray(bias.astype(np.float32)[None, :]),
            "iota_row": iota_row,
            "gidx": gidx,
            "eidx": eidx,
            "slotf": slotf,
            "eradd2": eradd2,
        })
    return in_maps, newid


def _build_program(ncores):
    import concourse.bass as bass
    import concourse.tile as tile
    from concourse import bacc, mybir
    from contextlib import ExitStack

    f32 = mybir.dt.float32
    bf16 = mybir.dt.bfloat16
    i16 = mybir.dt.int16

    sh = NPAD // ncores

    nc = bacc.Bacc("TRN2", target_bir_lowering=False, debug=False,
                   num_devices=ncores, num_swdge_queues=4)

    featsT = nc.dram_tensor("featsT", [D, sh], f32, kind="ExternalInput").ap()
    Wext = nc.dram_tensor("Wext", [D, D + 2], f32, kind="ExternalInput").ap()
    bias_in = nc.dram_tensor("bias_in", [1, D], f32, kind="ExternalInput").ap()
    iota_in = nc.dram_tensor("iota_row", [128, 128], f32,
                             kind="ExternalInput").ap()
    gidx_in = nc.dram_tensor("gidx", [128, NW * NG_W, GB // 16], i16,
                             kind="ExternalInput").ap()
    eidx_in = nc.dram_tensor("eidx", [128, NBATCH, NBW * GW // 16], i16,
                             kind="ExternalInput").ap()
    slotf_in = nc.dram_tensor("slotf", [128, NW * 3, 1], f32,
                              kind="ExternalInput").ap()
    eradd2_in = nc.dram_tensor("eradd2", [128, NW, 2], f32,
                               kind="ExternalInput").ap()
    out_ext = nc.dram_tensor("out", [sh, D], f32, kind="ExternalOutput").ap()
    red_dump = (nc.dram_tensor("red_dump", [128, NW * 3, RW], f32,
                               kind="ExternalOutput").ap()
                if DEBUG_RED else None)

    hstage = nc.dram_tensor("hstage", [sh, DA], bf16).ap()
    if ncores > 1:
        hfull = nc.dram_tensor("hfull", [NPAD, DA], bf16,
                               addr_space="Shared").ap()
    else:
        hfull = hstage

    NT = sh // 128  # node tiles per core

    with tile.TileContext(nc) as tc, ExitStack() as ctx:
        const = ctx.enter_context(tc.tile_pool(name="const", bufs=1))

        w_sb = const.tile([128, 2, D + 2], f32, tag="w_sb")
        nc.sync.dma_start(w_sb[:, 0, :], Wext[0:128, :])
        nc.sync.dma_start(w_sb[:, 1, :], Wext[128:256, :])
        iota_sb = const.tile([128, 128], f32, tag="iota")
        nc.sync.dma_start(iota_sb[:], iota_in[:])
        bias_row = const.tile([1, D], f32, tag="bias_row")
        nc.sync.dma_start(bias_row[:], bias_in[:])
        ones_col = const.tile([1, 128], f32, tag="ones_col")
        nc.vector.memset(ones_col[:], 1.0)
        gidx_sb = const.tile([128, NW * NG_W, GB // 16], i16, tag="gidx")
        nc.sync.dma_start(gidx_sb[:], gidx_in[:])
        eidx_sb = const.tile([128, NBATCH, NBW * GW // 16], i16, tag="eidx")
        nc.sync.dma_start(eidx_sb[:], eidx_in[:])
        eradd2_sb = const.tile([128, NW, 2], f32, tag="eradd2")
        nc.sync.dma_start(eradd2_sb[:], eradd2_in[:])

        bias_rep = const.tile([128, D], f32, tag="bias_rep")
        sel = const.tile([128, NW * 3, 128], f32, tag="sel")
        with tc.tile_pool(name="psum_b", bufs=1, space="PSUM") as psb, \
             tc.tile_pool(name="slotp", bufs=1) as slotp:
            pb = psb.tile([128, D], f32)
            nc.tensor.matmul(pb[:], lhsT=ones_col[:], rhs=bias_row[:],
                             start=True, stop=True)
            nc.vector.tensor_copy(bias_rep[:], pb[:])
            slotf_sb = slotp.tile([128, NW * 3, 1], f32, tag="slotf")
            nc.sync.dma_start(slotf_sb[:], slotf_in[:])
            nc.vector.tensor_tensor(
                out=sel[:],
                in0=iota_sb[:, None, :].broadcast_to([128, NW * 3, 128]),
                in1=slotf_sb[:].broadcast_to([128, NW * 3, 128]),
                op=mybir.AluOpType.is_equal,
            )

        # persistent per-rep tiles
        persist = ctx.enter_context(tc.tile_pool(name="persist", bufs=1))
        red = persist.tile([128, NW * 3, RW], f32, tag="red")
        ustage = persist.tile([128, NW, UW], f32, tag="ustage")

        reg_gb = nc.gpsimd.to_reg(GB)
        reg_eb = nc.gpsimd.to_reg(NBW * GW)

        for _rep in range(REPEAT):
            # ---------------- Phase 1: rows [h+bias | el | er] ------------
            with tc.tile_pool(name="p1", bufs=1) as p1, \
                 tc.tile_pool(name="p1ps", bufs=4, space="PSUM") as pp:
                ftT = p1.tile([128, 2, sh], f32, tag="ftT")
                nc.sync.dma_start(ftT[:, 0, :], featsT[0:128, :])
                nc.sync.dma_start(ftT[:, 1, :], featsT[128:256, :])
                hbig = p1.tile([128, NT, DA], bf16, tag="hbig")
                nc.vector.memset(hbig[:], 0.0)
                fbuf = p1.tile([128, NT, D + 2], f32, tag="fbuf")
                if ABLATE_P1:
                    pass
                else:
                    for nt in range(NT):
                        nsl = bass.ts(nt, 128)
                        hp = pp.tile([128, D + 2], f32, tag="hp")
                        for k in range(2):
                            nc.tensor.matmul(hp[:], lhsT=ftT[:, k, nsl],
                                             rhs=w_sb[:, k, :],
                                             start=(k == 0), stop=(k == 1))
                        nc.vector.tensor_copy(fbuf[:, nt, :], hp[:])
                    nc.vector.tensor_tensor(
                        out=hbig[:, :, 0:D], in0=fbuf[:, :, 0:D],
                        in1=bias_rep[:, None, :].broadcast_to([128, NT, D]),
                        op=mybir.AluOpType.add)
                    # el/er exact f32 via bitcast cols; el = -1e30 on fakes
                    nc.vector.tensor_tensor(
                        out=hbig[:, :, D:D + 4].bitcast(f32),
                        in0=fbuf[:, :, D:D + 2], in1=eradd2_sb[:],
                        op=mybir.AluOpType.add)
                nc.sync.dma_start(
                    hstage.rearrange("(t p) d -> p t d", p=128), hbig[:])

            if ncores > 1 and not ABLATE_COLL:
                nc.gpsimd.collective_compute(
                    "AllGather",
                    mybir.AluOpType.bypass,
                    replica_groups=[list(range(ncores))],
                    ins=[hstage[:]],
                    outs=[hfull[:]],
                )

            # ---------------- Phase 2: gather/weight/reduce ---------------
            with tc.tile_pool(name="edge", bufs=1) as ep, \
                 tc.tile_pool(name="small", bufs=2) as sp:
                for b in range(NBATCH):
                    ghr = ep.tile([128, NBW * CHW, DA], bf16, tag="ghr")
                    for k in range(NBW * NG_W):
                        nc.gpsimd.dma_gather(
                            out_ap=ghr[:, 8 * k:8 * k + 8, :],
                            in_ap=hfull[:],
                            idxs_ap=gidx_sb[:, b * NBW * NG_W + k, :],
                            num_idxs=GB, num_idxs_reg=reg_gb, elem_size=DA,
                        )
                    egr = ep.tile([128, NBW * 3, 128], bf16, tag="egr")
                    nc.gpsimd.dma_gather(
                        out_ap=egr[:],
                        in_ap=hfull[:, D:DA],
                        idxs_ap=eidx_sb[:, b, :],
                        num_idxs=NBW * GW, num_idxs_reg=reg_eb,
                        elem_size=128, elem_step=DA,
                    )
                    # t = el + er  [128, 6, 16, 1]
                    ghr_q = ghr[:].rearrange("p (q j) e -> p q j e", j=R)
                    elv = ghr_q[:, :, :, D:D + 2].bitcast(f32)
                    erv = egr[:, :, 2:4].bitcast(f32)   # [128, 6, 1]
                    t_sb = sp.tile([128, NBW * 3, R, 1], f32, tag="t")
                    nc.vector.tensor_tensor(
                        out=t_sb[:], in0=elv,
                        in1=erv[:, :, None, :].broadcast_to(
                            [128, NBW * 3, R, 1]),
                        op=mybir.AluOpType.add)
                    t2_sb = sp.tile([128, NBW * 3, R, 1], f32, tag="t2")
                    nc.vector.tensor_scalar_mul(t2_sb[:], t_sb[:], NEG_SLOPE)
                    lr_sb = sp.tile([128, NBW * 3, R, 1], f32, tag="lr")
                    nc.vector.tensor_tensor(
                        out=lr_sb[:], in0=t_sb[:], in1=t2_sb[:],
                        op=mybir.AluOpType.max)
                    w_sb2 = sp.tile([128, NBW * 3, R, 1], f32, tag="w")
                    nc.scalar.activation(
                        w_sb2[:], lr_sb[:], mybir.ActivationFunctionType.Exp)
                    nc.vector.tensor_tensor(
                        out=ghr_q[:, :, :, 0:D], in0=ghr_q[:, :, :, 0:D],
                        in1=w_sb2[:].broadcast_to([128, NBW * 3, R, D]),
                        op=mybir.AluOpType.mult)
                    nc.vector.tensor_copy(ghr_q[:, :, :, D:D + 1], w_sb2[:])
                    ghr_r = ghr[:, :, 0:RW].rearrange(
                        "p (q j) e -> p q e j", j=R)
                    nc.vector.tensor_reduce(
                        out=red[:, b * NBW * 3:(b + 1) * NBW * 3, :],
                        in_=ghr_r, axis=mybir.AxisListType.X,
                        op=mybir.AluOpType.add)

            if DEBUG_RED:
                nc.sync.dma_start(red_dump[:], red[:])

            # ---------------- Phase 3: Sel matmul + epilogue --------------
            with tc.tile_pool(name="mm", bufs=4, space="PSUM") as mp, \
                 tc.tile_pool(name="ot", bufs=1) as otp:
                for w in range(NW):
                    pu = mp.tile([128, UW], f32, tag="pu")
                    for k in range(3):
                        nc.tensor.matmul(
                            pu[:], lhsT=sel[:, 3 * w + k, :],
                            rhs=red[:, 3 * w + k, 0:UW],
                            start=(k == 0), stop=(k == 2))
                    nc.vector.tensor_copy(ustage[:, w, :], pu[:])
                rcp = otp.tile([128, NW, 1], f32, tag="rcp")
                nc.vector.reciprocal(rcp[:], ustage[:, :, D:D + 1])
                ot = otp.tile([128, NW, D], f32, tag="ot")
                nc.vector.tensor_tensor(
                    out=ot[:], in0=ustage[:, :, 0:D],
                    in1=rcp[:].broadcast_to([128, NW, D]),
                    op=mybir.AluOpType.mult)
                nc.sync.dma_start(
                    out_ext.rearrange("(w p) d -> p w d", p=128), ot[:])

    nc.compile()
    return nc


def _get_program(ncores):
    if ncores not in _prog_cache:
        _prog_cache[ncores] = _build_program(ncores)
    return _prog_cache[ncores]


def kernel(feats, W, attn_l, attn_r, bias, src, dst):
    from concourse.bass_utils import run_bass_kernel_spmd

    feats = np.asarray(feats, np.float32)
    W = np.asarray(W, np.float32)
    attn_l = np.asarray(attn_l, np.float32)
    attn_r = np.asarray(attn_r, np.float32)
    bias = np.asarray(bias, np.float32)
    src = np.asarray(src)
    dst = np.asarray(dst)

    in_maps, newid = _prep_inputs(feats, W, attn_l, attn_r, bias, src, dst)
    nc = _get_program(NCORES)
    res = run_bass_kernel_spmd(nc, in_maps, list(range(NCORES)))
    shards = [np.asarray(res.results[c]["out"]) for c in range(NCORES)]
    out_cat = np.concatenate(shards, axis=0)
    return out_cat[newid[:N]].astype(np.float32)


# revision 9
# speedup vs baseline: 1.1787x; 1.1787x over previous
"""Single-head GAT (DGL GATConv) forward on 8 Trainium2 NeuronCores — v3.

Cost model (measured): ~50-100us fixed per instruction regardless of size;
dma_gather moves <=1024 rows per instruction (SWDGE ucode cap);
dma_scatter_add races on duplicate indices on HW (unusable here).

Design: minimize instruction count.
  - Node rows [h+bias (256 bf16) | el f32 (bitcast 2) | er f32 (bitcast 2) |
    0 pad] = 384 bf16 (768 B), AllGathered to hfull [10240, 384].
  - Edges grouped per dst into R=16-slot groups (per-dst padding to 16);
    10 windows/core of 128 dst; per-window stream padded to 6144 slots
    (384 groups).  Stream gathered by src via dma_gather (1024 rows/op).
  - Per-edge weight w = exp(lrelu(el_src + er_dst)) on DVE/ACT in batches
    of 2 windows; rows weighted in place; R-reduce -> 384 group rows per
    window [Sum w*h | Sum w | junk] f32.
  - Group->dst scatter via small Sel matmul: 3 chunks of 128 groups per
    window, fp32 PE, psum [128, 257].
  - Epilogue: out = num/denom (one reciprocal + one multiply + one DMA).

kernel(**inputs) takes full unsharded inputs, returns [10000, 256] fp32.
"""

import numpy as np
import ml_dtypes

N = 10000
E = 320000
D = 256
NPAD = 10240
NCORES = 8
SH = NPAD // NCORES          # 1280 nodes per core
WINN = 128                   # dst nodes per window
NW = SH // WINN              # 10 windows per core
R = 16                       # edge slots per reduce-group
GW = 384                     # groups per window (3 chunks of 128)
SLOTS_W = GW * R             # 6144 edge slots per window
CHW = SLOTS_W // 128         # 48 gather-chunks per window
NBW = 2                      # windows per DVE batch
NBATCH = NW // NBW           # 5 batches
GB = 1024                    # rows per dma_gather
NG_W = SLOTS_W // GB         # 6 gathers per window
DA = 384                     # bf16 elements per node row (768 B)
UW = 257                     # used columns of reduced rows
RW = 320                     # reduced row width f32 (1280 B)
NEG_SLOPE = 0.2
REPEAT = 1
DEBUG_RED = False            # add a DRAM dump of the reduced group rows
ABLATE_GATHER = False        # timing: memset instead of dma_gather
ABLATE_MM = False            # timing: memset ustage instead of sel matmuls
ABLATE_DVE = False           # timing: memset red instead of weight chain
ABLATE_COLL = False          # timing: skip AllGather
ABLATE_P1 = False            # timing: memset hbig instead of h matmuls
NQ = 1                       # SWDGE queues to round-robin gathers over
USE_LRELU_ACT = False        # HW Lrelu ignores alpha; keep DVE lrelu

_BF16 = ml_dtypes.bfloat16

_prog_cache = {}


def _prep_inputs(feats, W, attn_l, attn_r, bias, src, dst, ncores=NCORES):
    """Host-side sharding/index prep. Returns (in_maps, newid)."""
    import heapq

    sh = NPAD // ncores
    nw_g = NPAD // WINN                       # 80 windows globally

    feats_pad = np.zeros((NPAD, D), np.float32)
    feats_pad[:N] = feats
    val = (W @ attn_l).astype(np.float32)
    var = (W @ attn_r).astype(np.float32)
    w_ext = np.concatenate(
        [W.astype(np.float32), val[:, None], var[:, None]], axis=1)  # [256,258]

    src = src.astype(np.int64)
    dst = dst.astype(np.int64)
    deg = np.bincount(dst, minlength=NPAD).astype(np.int64)
    gcount = -(-deg // R)                      # groups per node

    # heap-pack nodes into windows of 128, balancing group counts
    norder = np.argsort(-gcount, kind="stable")
    bin_groups = np.zeros(nw_g, np.int64)
    bin_count = np.zeros(nw_g, np.int64)
    newid = np.empty(NPAD, np.int64)
    heap = [(0, b) for b in range(nw_g)]
    heapq.heapify(heap)
    for n in norder:
        while True:
            _, b = heapq.heappop(heap)
            if bin_count[b] < WINN:
                break
        newid[n] = b * WINN + bin_count[b]
        bin_count[b] += 1
        bin_groups[b] += gcount[n]
        if bin_count[b] < WINN:
            heapq.heappush(heap, (bin_groups[b], b))
    assert bin_groups.max() <= GW, (bin_groups.max(), GW)
    inv = np.empty(NPAD, np.int64)
    inv[newid] = np.arange(NPAD)
    feats_pad = feats_pad[inv]                 # row j = new id j

    # one fake node id (h=0, el=0; er forced to -1e30 on device)
    fakes = newid[N:NPAD]
    assert len(fakes) > 0
    fid = int(fakes[0])
    is_fake = np.zeros(NPAD, bool)
    is_fake[fakes] = True

    n_src = newid[src]
    n_dst = newid[dst]
    order = np.argsort(n_dst, kind="stable")
    s_src = n_src[order]
    s_dst = n_dst[order]
    starts = np.searchsorted(s_dst, np.arange(NPAD + 1))

    # per-window edge stream (6144 slots) + group table (384 groups)
    gath = np.full((nw_g, SLOTS_W), fid, np.int64)   # src gather idx
    gslot = np.full((nw_g, GW), -1.0, np.float32)    # group -> dst slot
    gdst = np.full((nw_g, GW), fid, np.int64)        # group -> dst node id
    for w in range(nw_g):
        pos = 0
        gi = 0
        for s in range(WINN):
            node = w * WINN + s
            a, b = starts[node], starts[node + 1]
            k = b - a
            if k == 0:
                continue
            ng = -(-k // R)
            gath[w, pos:pos + k] = s_src[a:b]
            gslot[w, gi:gi + ng] = s
            gdst[w, gi:gi + ng] = node
            pos += ng * R
            gi += ng
        assert gi <= GW and pos <= SLOTS_W

    # stream slot i of window w -> token (p = i%128, chunk c = i//128);
    # group of token = (p, c//R); member j = c%R.
    # Slot numbering inside window must place group g's members at
    # chunks [ (g//128)*R , +R ) on partition g%128.  Build mapping:
    # linear group gi -> (p=gi%128, q=gi//128); member j -> stream pos
    # p + 128*(q*R + j).
    lin = np.arange(SLOTS_W)
    gi_of = (lin // R)                       # linear group id 0..383
    j_of = lin % R
    p_of = gi_of % 128
    q_of = gi_of // 128
    tok = p_of + 128 * (q_of * R + j_of)     # token position in window
    # gather index vector in token order: gtok[w, tok] = gath[w, lin]
    gtok = np.empty_like(gath)
    gtok[:, tok] = gath

    def wrap_idx(flat):
        # [.., n] -> [128, .., n//16] int16 (i -> partition i%16, col i//16)
        nn = flat.shape[-1]
        w16 = flat.reshape(*flat.shape[:-1], nn // 16, 16)
        w16 = np.moveaxis(w16, -1, 0)                  # [16, .., n//16]
        rep = np.tile(w16, (8,) + (1,) * (w16.ndim - 1))
        return np.ascontiguousarray(rep.astype(np.int16))

    iota_row = np.ascontiguousarray(
        np.broadcast_to(np.arange(128, dtype=np.float32), (128, 128)))

    in_maps = []
    for c in range(ncores):
        wlo, whi = c * NW, c * NW + NW
        featsT = np.ascontiguousarray(feats_pad[c * sh:(c + 1) * sh].T)
        # gather idx: per core stream = concat of its windows' token streams
        stream = gtok[wlo:whi].reshape(-1)             # [61440]
        gidx = wrap_idx(stream.reshape(NW * NG_W, GB)) # [128, 60, 64]
        # er gather idx: per batch 768 groups (2 windows)
        gd = gdst[wlo:whi].reshape(NBATCH, NBW * GW)   # [5, 768]
        eidx = wrap_idx(gd)                            # [128, 5, 48]
        # slotf: [128, 30, 128]: chunk 3w+k partition p -> slot of group
        sl = gslot[wlo:whi].reshape(NW * 3, 128).T     # [128, 30]
        slotf = np.ascontiguousarray(
            np.broadcast_to(sl[:, :, None], (128, NW * 3, 1)).astype(
                np.float32))
        # eradd2 [128, 10, 2]: fake nodes get el = -1e30 so pad slots
        # (which gather the fake row by src but take er from their REAL
        # group dst) still produce w = exp(lrelu(-1e30 + er)) = 0.
        lid = (np.arange(NW)[None, :] * 128
               + np.arange(128)[:, None])              # [128, 10] local id
        fk = is_fake[c * sh + lid]
        eradd2 = np.zeros((128, NW, 2), np.float32)
        eradd2[:, :, 0] = np.where(fk, -1e30, 0.0)
        in_maps.append({
            "featsT": featsT,
            "Wext": w_ext,
            "bias_in": np.ascontiguousarray(bias.astype(np.float32)[None, :]),
            "iota_row": iota_row,
            "gidx": gidx,
            "eidx": eidx,
            "slotf": slotf,
            "eradd2": eradd2,
        })
    return in_maps, newid


def _build_program(ncores):
    import concourse.bass as bass
    import concourse.tile as tile
    from concourse import bacc, mybir
    from contextlib import ExitStack

    f32 = mybir.dt.float32
    bf16 = mybir.dt.bfloat16
    i16 = mybir.dt.int16

    sh = NPAD // ncores

    nc = bacc.Bacc("TRN2", target_bir_lowering=False, debug=False,
                   num_devices=ncores, num_swdge_queues=4)

    featsT = nc.dram_tensor("featsT", [D, sh], f32, kind="ExternalInput").ap()
    Wext = nc.dram_tensor("Wext", [D, D + 2], f32, kind="ExternalInput").ap()
    bias_in = nc.dram_tensor("bias_in", [1, D], f32, kind="ExternalInput").ap()
    iota_in = nc.dram_tensor("iota_row", [128, 128], f32,
                             kind="ExternalInput").ap()
    gidx_in = nc.dram_tensor("gidx", [128, NW * NG_W, GB // 16], i16,
                             kind="ExternalInput").ap()
    eidx_in = nc.dram_tensor("eidx", [128, NBATCH, NBW * GW // 16], i16,
                             kind="ExternalInput").ap()
    slotf_in = nc.dram_tensor("slotf", [128, NW * 3, 1], f32,
                              kind="ExternalInput").ap()
    eradd2_in = nc.dram_tensor("eradd2", [128, NW, 2], f32,
                               kind="ExternalInput").ap()
    out_ext = nc.dram_tensor("out", [sh, D], f32, kind="ExternalOutput").ap()
    red_dump = (nc.dram_tensor("red_dump", [128, NW * 3, RW], f32,
                               kind="ExternalOutput").ap()
                if DEBUG_RED else None)

    hstage = nc.dram_tensor("hstage", [sh, DA], bf16).ap()
    if ncores > 1:
        hfull = nc.dram_tensor("hfull", [NPAD, DA], bf16,
                               addr_space="Shared").ap()
    else:
        hfull = hstage

    NT = sh // 128  # node tiles per core

    with tile.TileContext(nc) as tc, ExitStack() as ctx:
        const = ctx.enter_context(tc.tile_pool(name="const", bufs=1))

        w_sb = const.tile([128, 2, D + 2], f32, tag="w_sb")
        nc.sync.dma_start(w_sb[:, 0, :], Wext[0:128, :])
        nc.sync.dma_start(w_sb[:, 1, :], Wext[128:256, :])
        iota_sb = const.tile([128, 128], f32, tag="iota")
        nc.sync.dma_start(iota_sb[:], iota_in[:])
        bias_row = const.tile([1, D], f32, tag="bias_row")
        nc.sync.dma_start(bias_row[:], bias_in[:])
        ones_col = const.tile([1, 128], f32, tag="ones_col")
        nc.vector.memset(ones_col[:], 1.0)
        gidx_sb = const.tile([128, NW * NG_W, GB // 16], i16, tag="gidx")
        nc.sync.dma_start(gidx_sb[:], gidx_in[:])
        eidx_sb = const.tile([128, NBATCH, NBW * GW // 16], i16, tag="eidx")
        nc.sync.dma_start(eidx_sb[:], eidx_in[:])
        eradd2_sb = const.tile([128, NW, 2], f32, tag="eradd2")
        nc.sync.dma_start(eradd2_sb[:], eradd2_in[:])

        bias_rep = const.tile([128, D], f32, tag="bias_rep")
        sel = const.tile([128, NW * 3, 128], f32, tag="sel")
        with tc.tile_pool(name="psum_b", bufs=1, space="PSUM") as psb, \
             tc.tile_pool(name="slotp", bufs=1) as slotp:
            pb = psb.tile([128, D], f32)
            nc.tensor.matmul(pb[:], lhsT=ones_col[:], rhs=bias_row[:],
                             start=True, stop=True)
            nc.vector.tensor_copy(bias_rep[:], pb[:])
            slotf_sb = slotp.tile([128, NW * 3, 1], f32, tag="slotf")
            nc.sync.dma_start(slotf_sb[:], slotf_in[:])
            nc.vector.tensor_tensor(
                out=sel[:],
                in0=iota_sb[:, None, :].broadcast_to([128, NW * 3, 128]),
                in1=slotf_sb[:].broadcast_to([128, NW * 3, 128]),
                op=mybir.AluOpType.is_equal,
            )

        # persistent per-rep tiles
        persist = ctx.enter_context(tc.tile_pool(name="persist", bufs=1))
        red = persist.tile([128, NW * 3, RW], f32, tag="red")
        ustage = persist.tile([128, NW, UW], f32, tag="ustage")

        reg_gb = nc.gpsimd.to_reg(GB)
        reg_eb = nc.gpsimd.to_reg(NBW * GW)

        for _rep in range(REPEAT):
            # ---------------- Phase 1: rows [h+bias | el | er] ------------
            with tc.tile_pool(name="p1", bufs=1) as p1, \
                 tc.tile_pool(name="p1ps", bufs=4, space="PSUM") as pp:
                ftT = p1.tile([128, 2, sh], f32, tag="ftT")
                nc.sync.dma_start(ftT[:, 0, :], featsT[0:128, :])
                nc.sync.dma_start(ftT[:, 1, :], featsT[128:256, :])
                hbig = p1.tile([128, NT, DA], bf16, tag="hbig")
                nc.vector.memset(hbig[:], 0.0)
                fbuf = p1.tile([128, NT, D + 2], f32, tag="fbuf")
                if ABLATE_P1:
                    pass
                else:
                    for nt in range(NT):
                        nsl = bass.ts(nt, 128)
                        hp = pp.tile([128, D + 2], f32, tag="hp")
                        for k in range(2):
                            nc.tensor.matmul(hp[:], lhsT=ftT[:, k, nsl],
                                             rhs=w_sb[:, k, :],
                                             start=(k == 0), stop=(k == 1))
                        nc.vector.tensor_copy(fbuf[:, nt, :], hp[:])
                    nc.vector.tensor_tensor(
                        out=hbig[:, :, 0:D], in0=fbuf[:, :, 0:D],
                        in1=bias_rep[:, None, :].broadcast_to([128, NT, D]),
                        op=mybir.AluOpType.add)
                    # el/er exact f32 via bitcast cols; el = -1e30 on fakes
                    nc.vector.tensor_tensor(
                        out=hbig[:, :, D:D + 4].bitcast(f32),
                        in0=fbuf[:, :, D:D + 2], in1=eradd2_sb[:],
                        op=mybir.AluOpType.add)
                nc.sync.dma_start(
                    hstage.rearrange("(t p) d -> p t d", p=128), hbig[:])

            if ncores > 1 and not ABLATE_COLL:
                nc.gpsimd.collective_compute(
                    "AllGather",
                    mybir.AluOpType.bypass,
                    replica_groups=[list(range(ncores))],
                    ins=[hstage[:]],
                    outs=[hfull[:]],
                )

            # ---------------- Phase 2: gather/weight/reduce ---------------
            with tc.tile_pool(name="edge", bufs=1) as ep, \
                 tc.tile_pool(name="small", bufs=2) as sp:
                for b in range(NBATCH):
                    ghr = ep.tile([128, NBW * CHW, DA], bf16, tag="ghr")
                    egr = ep.tile([128, NBW * 3, 128], bf16, tag="egr")
                    if ABLATE_GATHER:
                        nc.vector.memset(ghr[:], 0.01)
                        nc.vector.memset(egr[:], 0.01)
                    else:
                        for k in range(NBW * NG_W):
                            nc.gpsimd.dma_gather(
                                out_ap=ghr[:, 8 * k:8 * k + 8, :],
                                in_ap=hfull[:],
                                idxs_ap=gidx_sb[:, b * NBW * NG_W + k, :],
                                num_idxs=GB, num_idxs_reg=reg_gb,
                                elem_size=DA, queue_num=k % NQ,
                            )
                        nc.gpsimd.dma_gather(
                            out_ap=egr[:],
                            in_ap=hfull[:, D:DA],
                            idxs_ap=eidx_sb[:, b, :],
                            num_idxs=NBW * GW, num_idxs_reg=reg_eb,
                            elem_size=128, elem_step=DA,
                        )
                    if ABLATE_DVE:
                        nc.vector.memset(
                            red[:, b * NBW * 3:(b + 1) * NBW * 3, :], 0.02)
                        continue
                    # t = el + er  [128, 6, 16, 1]
                    ghr_q = ghr[:].rearrange("p (q j) e -> p q j e", j=R)
                    elv = ghr_q[:, :, :, D:D + 2].bitcast(f32)
                    erv = egr[:, :, 2:4].bitcast(f32)   # [128, 6, 1]
                    t_sb = sp.tile([128, NBW * 3, R, 1], f32, tag="t")
                    nc.vector.tensor_tensor(
                        out=t_sb[:], in0=elv,
                        in1=erv[:, :, None, :].broadcast_to(
                            [128, NBW * 3, R, 1]),
                        op=mybir.AluOpType.add)
                    lr_sb = sp.tile([128, NBW * 3, R, 1], f32, tag="lr")
                    if USE_LRELU_ACT:
                        nc.scalar.activation(
                            lr_sb[:], t_sb[:],
                            mybir.ActivationFunctionType.Lrelu,
                            alpha=NEG_SLOPE)
                    else:
                        t2_sb = sp.tile([128, NBW * 3, R, 1], f32, tag="t2")
                        nc.vector.tensor_scalar_mul(t2_sb[:], t_sb[:],
                                                    NEG_SLOPE)
                        nc.vector.tensor_tensor(
                            out=lr_sb[:], in0=t_sb[:], in1=t2_sb[:],
                            op=mybir.AluOpType.max)
                    w_sb2 = sp.tile([128, NBW * 3, R, 1], f32, tag="w")
                    nc.scalar.activation(
                        w_sb2[:], lr_sb[:], mybir.ActivationFunctionType.Exp)
                    nc.vector.tensor_tensor(
                        out=ghr_q[:, :, :, 0:D], in0=ghr_q[:, :, :, 0:D],
                        in1=w_sb2[:].broadcast_to([128, NBW * 3, R, D]),
                        op=mybir.AluOpType.mult)
                    nc.vector.tensor_copy(ghr_q[:, :, :, D:D + 1], w_sb2[:])
                    ghr_r = ghr[:, :, 0:RW].rearrange(
                        "p (q j) e -> p q e j", j=R)
                    nc.vector.tensor_reduce(
                        out=red[:, b * NBW * 3:(b + 1) * NBW * 3, :],
                        in_=ghr_r, axis=mybir.AxisListType.X,
                        op=mybir.AluOpType.add)

            if DEBUG_RED:
                nc.sync.dma_start(red_dump[:], red[:])

            # ---------------- Phase 3: Sel matmul + epilogue --------------
            with tc.tile_pool(name="mm", bufs=4, space="PSUM") as mp, \
                 tc.tile_pool(name="ot", bufs=1) as otp:
                if ABLATE_MM:
                    nc.vector.memset(ustage[:], 0.25)
                else:
                    for w in range(NW):
                        pu = mp.tile([128, UW], f32, tag="pu")
                        for k in range(3):
                            nc.tensor.matmul(
                                pu[:], lhsT=sel[:, 3 * w + k, :],
                                rhs=red[:, 3 * w + k, 0:UW],
                                start=(k == 0), stop=(k == 2))
                        nc.vector.tensor_copy(ustage[:, w, :], pu[:])
                rcp = otp.tile([128, NW, 1], f32, tag="rcp")
                nc.vector.reciprocal(rcp[:], ustage[:, :, D:D + 1])
                ot = otp.tile([128, NW, D], f32, tag="ot")
                nc.vector.tensor_tensor(
                    out=ot[:], in0=ustage[:, :, 0:D],
                    in1=rcp[:].broadcast_to([128, NW, D]),
                    op=mybir.AluOpType.mult)
                nc.sync.dma_start(
                    out_ext.rearrange("(w p) d -> p w d", p=128), ot[:])

    nc.compile()
    return nc


def _get_program(ncores):
    if ncores not in _prog_cache:
        _prog_cache[ncores] = _build_program(ncores)
    return _prog_cache[ncores]


def kernel(feats, W, attn_l, attn_r, bias, src, dst):
    from concourse.bass_utils import run_bass_kernel_spmd

    feats = np.asarray(feats, np.float32)
    W = np.asarray(W, np.float32)
    attn_l = np.asarray(attn_l, np.float32)
    attn_r = np.asarray(attn_r, np.float32)
    bias = np.asarray(bias, np.float32)
    src = np.asarray(src)
    dst = np.asarray(dst)

    in_maps, newid = _prep_inputs(feats, W, attn_l, attn_r, bias, src, dst)
    nc = _get_program(NCORES)
    res = run_bass_kernel_spmd(nc, in_maps, list(range(NCORES)))
    shards = [np.asarray(res.results[c]["out"]) for c in range(NCORES)]
    out_cat = np.concatenate(shards, axis=0)
    return out_cat[newid[:N]].astype(np.float32)


# revision 13
# speedup vs baseline: 1.2160x; 1.0317x over previous
"""Single-head GAT (DGL GATConv) forward on 8 Trainium2 NeuronCores — v3.

Cost model (measured): ~50-100us fixed per instruction regardless of size;
dma_gather moves <=1024 rows per instruction (SWDGE ucode cap);
dma_scatter_add races on duplicate indices on HW (unusable here).

Design: minimize instruction count.
  - Node rows [h+bias (256 bf16) | el f32 (bitcast 2) | er f32 (bitcast 2) |
    0 pad] = 384 bf16 (768 B), AllGathered to hfull [10240, 384].
  - Edges grouped per dst into R=16-slot groups (per-dst padding to 16);
    10 windows/core of 128 dst; per-window stream padded to 6144 slots
    (384 groups).  Stream gathered by src via dma_gather (1024 rows/op).
  - Per-edge weight w = exp(lrelu(el_src + er_dst)) on DVE/ACT in batches
    of 2 windows; rows weighted in place; R-reduce -> 384 group rows per
    window [Sum w*h | Sum w | junk] f32.
  - Group->dst scatter via small Sel matmul: 3 chunks of 128 groups per
    window, fp32 PE, psum [128, 257].
  - Epilogue: out = num/denom (one reciprocal + one multiply + one DMA).

kernel(**inputs) takes full unsharded inputs, returns [10000, 256] fp32.
"""

import numpy as np
import ml_dtypes

N = 10000
E = 320000
D = 256
NPAD = 10240
NCORES = 8
SH = NPAD // NCORES          # 1280 nodes per core
WINN = 128                   # dst nodes per window
NW = SH // WINN              # 10 windows per core
R = 16                       # edge slots per reduce-group
GW = 384                     # groups per window (3 chunks of 128)
SLOTS_W = GW * R             # 6144 edge slots per window
CHW = SLOTS_W // 128         # 48 gather-chunks per window
NBW = 2                      # windows per DVE batch
NBATCH = NW // NBW           # 5 batches
GB = 1024                    # rows per dma_gather
NG_W = SLOTS_W // GB         # 6 gathers per window
DA = 384                     # bf16 elements per node row (768 B)
UW = 257                     # used columns of reduced rows
RW = 320                     # reduced row width f32 (1280 B)
NEG_SLOPE = 0.2
REPEAT = 1
DEBUG_RED = False            # add a DRAM dump of the reduced group rows
ABLATE_GATHER = False        # timing: memset instead of dma_gather
ABLATE_MM = False            # timing: memset ustage instead of sel matmuls
ABLATE_DVE = False           # timing: memset red instead of weight chain
ABLATE_COLL = False          # timing: skip AllGather
ABLATE_P1 = False            # timing: memset hbig instead of h matmuls
NQ = 1                       # SWDGE queues to round-robin gathers over
USE_LRELU_ACT = False        # HW Lrelu ignores alpha; keep DVE lrelu
EDGE_BUFS = 1                # double-buffer ghr/egr for overlap
INTERLEAVE_MM = False        # emit sel matmuls inside the batch loop
SINGLE_PACKET = True         # dma_gather packetization mode

_BF16 = ml_dtypes.bfloat16

_prog_cache = {}


def _prep_inputs(feats, W, attn_l, attn_r, bias, src, dst, ncores=NCORES):
    """Host-side sharding/index prep. Returns (in_maps, newid)."""
    import heapq

    sh = NPAD // ncores
    nw_g = NPAD // WINN                       # 80 windows globally

    feats_pad = np.zeros((NPAD, D), np.float32)
    feats_pad[:N] = feats
    val = (W @ attn_l).astype(np.float32)
    var = (W @ attn_r).astype(np.float32)
    w_ext = np.concatenate(
        [W.astype(np.float32), val[:, None], var[:, None]], axis=1)  # [256,258]

    src = src.astype(np.int64)
    dst = dst.astype(np.int64)
    deg = np.bincount(dst, minlength=NPAD).astype(np.int64)
    gcount = -(-deg // R)                      # groups per node

    # heap-pack nodes into windows of 128, balancing group counts
    norder = np.argsort(-gcount, kind="stable")
    bin_groups = np.zeros(nw_g, np.int64)
    bin_count = np.zeros(nw_g, np.int64)
    newid = np.empty(NPAD, np.int64)
    heap = [(0, b) for b in range(nw_g)]
    heapq.heapify(heap)
    for n in norder:
        while True:
            _, b = heapq.heappop(heap)
            if bin_count[b] < WINN:
                break
        newid[n] = b * WINN + bin_count[b]
        bin_count[b] += 1
        bin_groups[b] += gcount[n]
        if bin_count[b] < WINN:
            heapq.heappush(heap, (bin_groups[b], b))
    assert bin_groups.max() <= GW, (bin_groups.max(), GW)
    inv = np.empty(NPAD, np.int64)
    inv[newid] = np.arange(NPAD)
    feats_pad = feats_pad[inv]                 # row j = new id j

    # one fake node id (h=0, el=0; er forced to -1e30 on device)
    fakes = newid[N:NPAD]
    assert len(fakes) > 0
    fid = int(fakes[0])
    is_fake = np.zeros(NPAD, bool)
    is_fake[fakes] = True

    n_src = newid[src]
    n_dst = newid[dst]
    order = np.argsort(n_dst, kind="stable")
    s_src = n_src[order]
    s_dst = n_dst[order]
    starts = np.searchsorted(s_dst, np.arange(NPAD + 1))

    # per-window edge stream (6144 slots) + group table (384 groups)
    gath = np.full((nw_g, SLOTS_W), fid, np.int64)   # src gather idx
    gslot = np.full((nw_g, GW), -1.0, np.float32)    # group -> dst slot
    gdst = np.full((nw_g, GW), fid, np.int64)        # group -> dst node id
    for w in range(nw_g):
        pos = 0
        gi = 0
        for s in range(WINN):
            node = w * WINN + s
            a, b = starts[node], starts[node + 1]
            k = b - a
            if k == 0:
                continue
            ng = -(-k // R)
            gath[w, pos:pos + k] = s_src[a:b]
            gslot[w, gi:gi + ng] = s
            gdst[w, gi:gi + ng] = node
            pos += ng * R
            gi += ng
        assert gi <= GW and pos <= SLOTS_W

    # stream slot i of window w -> token (p = i%128, chunk c = i//128);
    # group of token = (p, c//R); member j = c%R.
    # Slot numbering inside window must place group g's members at
    # chunks [ (g//128)*R , +R ) on partition g%128.  Build mapping:
    # linear group gi -> (p=gi%128, q=gi//128); member j -> stream pos
    # p + 128*(q*R + j).
    lin = np.arange(SLOTS_W)
    gi_of = (lin // R)                       # linear group id 0..383
    j_of = lin % R
    p_of = gi_of % 128
    q_of = gi_of // 128
    tok = p_of + 128 * (q_of * R + j_of)     # token position in window
    # gather index vector in token order: gtok[w, tok] = gath[w, lin]
    gtok = np.empty_like(gath)
    gtok[:, tok] = gath

    def wrap_idx(flat):
        # [.., n] -> [128, .., n//16] int16 (i -> partition i%16, col i//16)
        nn = flat.shape[-1]
        w16 = flat.reshape(*flat.shape[:-1], nn // 16, 16)
        w16 = np.moveaxis(w16, -1, 0)                  # [16, .., n//16]
        rep = np.tile(w16, (8,) + (1,) * (w16.ndim - 1))
        return np.ascontiguousarray(rep.astype(np.int16))

    iota_row = np.ascontiguousarray(
        np.broadcast_to(np.arange(128, dtype=np.float32), (128, 128)))

    in_maps = []
    for c in range(ncores):
        wlo, whi = c * NW, c * NW + NW
        featsT = np.ascontiguousarray(feats_pad[c * sh:(c + 1) * sh].T)
        # gather idx: per core stream = concat of its windows' token streams
        stream = gtok[wlo:whi].reshape(-1)             # [61440]
        gidx = wrap_idx(stream.reshape(NW * NG_W, GB)) # [128, 60, 64]
        # er gather idx: per batch 768 groups (2 windows)
        gd = gdst[wlo:whi].reshape(NBATCH, NBW * GW)   # [5, 768]
        eidx = wrap_idx(gd)                            # [128, 5, 48]
        # slotf: [128, 30, 128]: chunk 3w+k partition p -> slot of group
        sl = gslot[wlo:whi].reshape(NW * 3, 128).T     # [128, 30]
        slotf = np.ascontiguousarray(
            np.broadcast_to(sl[:, :, None], (128, NW * 3, 1)).astype(
                np.float32))
        # eradd2 [128, 10, 2]: fake nodes get el = -1e30 so pad slots
        # (which gather the fake row by src but take er from their REAL
        # group dst) still produce w = exp(lrelu(-1e30 + er)) = 0.
        lid = (np.arange(NW)[None, :] * 128
               + np.arange(128)[:, None])              # [128, 10] local id
        fk = is_fake[c * sh + lid]
        eradd2 = np.zeros((128, NW, 2), np.float32)
        eradd2[:, :, 0] = np.where(fk, -1e30, 0.0)
        in_maps.append({
            "featsT": featsT,
            "Wext": w_ext,
            "bias_in": np.ascontiguousarray(bias.astype(np.float32)[None, :]),
            "iota_row": iota_row,
            "gidx": gidx,
            "eidx": eidx,
            "slotf": slotf,
            "eradd2": eradd2,
        })
    return in_maps, newid


def _build_program(ncores):
    import concourse.bass as bass
    import concourse.tile as tile
    from concourse import bacc, mybir
    from contextlib import ExitStack

    f32 = mybir.dt.float32
    bf16 = mybir.dt.bfloat16
    i16 = mybir.dt.int16

    sh = NPAD // ncores

    nc = bacc.Bacc("TRN2", target_bir_lowering=False, debug=False,
                   num_devices=ncores, num_swdge_queues=4)

    featsT = nc.dram_tensor("featsT", [D, sh], f32, kind="ExternalInput").ap()
    Wext = nc.dram_tensor("Wext", [D, D + 2], f32, kind="ExternalInput").ap()
    bias_in = nc.dram_tensor("bias_in", [1, D], f32, kind="ExternalInput").ap()
    iota_in = nc.dram_tensor("iota_row", [128, 128], f32,
                             kind="ExternalInput").ap()
    gidx_in = nc.dram_tensor("gidx", [128, NW * NG_W, GB // 16], i16,
                             kind="ExternalInput").ap()
    eidx_in = nc.dram_tensor("eidx", [128, NBATCH, NBW * GW // 16], i16,
                             kind="ExternalInput").ap()
    slotf_in = nc.dram_tensor("slotf", [128, NW * 3, 1], f32,
                              kind="ExternalInput").ap()
    eradd2_in = nc.dram_tensor("eradd2", [128, NW, 2], f32,
                               kind="ExternalInput").ap()
    out_ext = nc.dram_tensor("out", [sh, D], f32, kind="ExternalOutput").ap()
    red_dump = (nc.dram_tensor("red_dump", [128, NW * 3, RW], f32,
                               kind="ExternalOutput").ap()
                if DEBUG_RED else None)

    hstage = nc.dram_tensor("hstage", [sh, DA], bf16).ap()
    if ncores > 1:
        hfull = nc.dram_tensor("hfull", [NPAD, DA], bf16,
                               addr_space="Shared").ap()
    else:
        hfull = hstage

    NT = sh // 128  # node tiles per core

    with tile.TileContext(nc) as tc, ExitStack() as ctx:
        const = ctx.enter_context(tc.tile_pool(name="const", bufs=1))

        w_sb = const.tile([128, 2, D + 2], f32, tag="w_sb")
        nc.sync.dma_start(w_sb[:, 0, :], Wext[0:128, :])
        nc.sync.dma_start(w_sb[:, 1, :], Wext[128:256, :])
        iota_sb = const.tile([128, 128], f32, tag="iota")
        nc.sync.dma_start(iota_sb[:], iota_in[:])
        bias_row = const.tile([1, D], f32, tag="bias_row")
        nc.sync.dma_start(bias_row[:], bias_in[:])
        ones_col = const.tile([1, 128], f32, tag="ones_col")
        nc.vector.memset(ones_col[:], 1.0)
        gidx_sb = const.tile([128, NW * NG_W, GB // 16], i16, tag="gidx")
        nc.sync.dma_start(gidx_sb[:], gidx_in[:])
        eidx_sb = const.tile([128, NBATCH, NBW * GW // 16], i16, tag="eidx")
        nc.sync.dma_start(eidx_sb[:], eidx_in[:])
        eradd2_sb = const.tile([128, NW, 2], f32, tag="eradd2")
        nc.sync.dma_start(eradd2_sb[:], eradd2_in[:])

        bias_rep = const.tile([128, D], f32, tag="bias_rep")
        sel = const.tile([128, NW * 3, 128], f32, tag="sel")
        with tc.tile_pool(name="psum_b", bufs=1, space="PSUM") as psb, \
             tc.tile_pool(name="slotp", bufs=1) as slotp:
            pb = psb.tile([128, D], f32)
            nc.tensor.matmul(pb[:], lhsT=ones_col[:], rhs=bias_row[:],
                             start=True, stop=True)
            nc.vector.tensor_copy(bias_rep[:], pb[:])
            slotf_sb = slotp.tile([128, NW * 3, 1], f32, tag="slotf")
            nc.sync.dma_start(slotf_sb[:], slotf_in[:])
            nc.vector.tensor_tensor(
                out=sel[:],
                in0=iota_sb[:, None, :].broadcast_to([128, NW * 3, 128]),
                in1=slotf_sb[:].broadcast_to([128, NW * 3, 128]),
                op=mybir.AluOpType.is_equal,
            )

        # persistent per-rep tiles
        persist = ctx.enter_context(tc.tile_pool(name="persist", bufs=1))
        red = persist.tile([128, NW * 3, RW], f32, tag="red")
        ustage = persist.tile([128, NW, UW], f32, tag="ustage")

        reg_gb = nc.gpsimd.to_reg(GB)
        reg_eb = nc.gpsimd.to_reg(NBW * GW)

        for _rep in range(REPEAT):
            # ---------------- Phase 1: rows [h+bias | el | er] ------------
            with tc.tile_pool(name="p1", bufs=1) as p1, \
                 tc.tile_pool(name="p1ps", bufs=4, space="PSUM") as pp:
                ftT = p1.tile([128, 2, sh], f32, tag="ftT")
                nc.sync.dma_start(ftT[:, 0, :], featsT[0:128, :])
                nc.sync.dma_start(ftT[:, 1, :], featsT[128:256, :])
                hbig = p1.tile([128, NT, DA], bf16, tag="hbig")
                nc.vector.memset(hbig[:], 0.0)
                fbuf = p1.tile([128, NT, D + 2], f32, tag="fbuf")
                if ABLATE_P1:
                    pass
                else:
                    for nt in range(NT):
                        nsl = bass.ts(nt, 128)
                        hp = pp.tile([128, D + 2], f32, tag="hp")
                        for k in range(2):
                            nc.tensor.matmul(hp[:], lhsT=ftT[:, k, nsl],
                                             rhs=w_sb[:, k, :],
                                             start=(k == 0), stop=(k == 1))
                        nc.vector.tensor_copy(fbuf[:, nt, :], hp[:])
                    nc.vector.tensor_tensor(
                        out=hbig[:, :, 0:D], in0=fbuf[:, :, 0:D],
                        in1=bias_rep[:, None, :].broadcast_to([128, NT, D]),
                        op=mybir.AluOpType.add)
                    # el/er exact f32 via bitcast cols; el = -1e30 on fakes
                    nc.vector.tensor_tensor(
                        out=hbig[:, :, D:D + 4].bitcast(f32),
                        in0=fbuf[:, :, D:D + 2], in1=eradd2_sb[:],
                        op=mybir.AluOpType.add)
                nc.sync.dma_start(
                    hstage.rearrange("(t p) d -> p t d", p=128), hbig[:])

            if ncores > 1 and not ABLATE_COLL:
                nc.gpsimd.collective_compute(
                    "AllGather",
                    mybir.AluOpType.bypass,
                    replica_groups=[list(range(ncores))],
                    ins=[hstage[:]],
                    outs=[hfull[:]],
                )

            # ---------------- Phase 2: gather/weight/reduce ---------------
            with tc.tile_pool(name="edge", bufs=EDGE_BUFS) as ep, \
                 tc.tile_pool(name="mm", bufs=4, space="PSUM") as mp, \
                 tc.tile_pool(name="small", bufs=2) as sp:
                for b in range(NBATCH):
                    ghr = ep.tile([128, NBW * CHW, DA], bf16, tag="ghr")
                    egr = ep.tile([128, NBW * 3, 128], bf16, tag="egr")
                    if ABLATE_GATHER:
                        nc.vector.memset(ghr[:], 0.01)
                        nc.vector.memset(egr[:], 0.01)
                    else:
                        for k in range(NBW * NG_W):
                            nc.gpsimd.dma_gather(
                                out_ap=ghr[:, 8 * k:8 * k + 8, :],
                                in_ap=hfull[:],
                                idxs_ap=gidx_sb[:, b * NBW * NG_W + k, :],
                                num_idxs=GB, num_idxs_reg=reg_gb,
                                elem_size=DA, queue_num=k % NQ,
                                single_packet=SINGLE_PACKET,
                            )
                        nc.gpsimd.dma_gather(
                            out_ap=egr[:],
                            in_ap=hfull[:, D:DA],
                            idxs_ap=eidx_sb[:, b, :],
                            num_idxs=NBW * GW, num_idxs_reg=reg_eb,
                            elem_size=128, elem_step=DA,
                        )
                    if ABLATE_DVE:
                        nc.vector.memset(
                            red[:, b * NBW * 3:(b + 1) * NBW * 3, :], 0.02)
                        continue
                    # t = el + er  [128, 6, 16, 1]
                    ghr_q = ghr[:].rearrange("p (q j) e -> p q j e", j=R)
                    elv = ghr_q[:, :, :, D:D + 2].bitcast(f32)
                    erv = egr[:, :, 2:4].bitcast(f32)   # [128, 6, 1]
                    t_sb = sp.tile([128, NBW * 3, R, 1], f32, tag="t")
                    nc.vector.tensor_tensor(
                        out=t_sb[:], in0=elv,
                        in1=erv[:, :, None, :].broadcast_to(
                            [128, NBW * 3, R, 1]),
                        op=mybir.AluOpType.add)
                    lr_sb = sp.tile([128, NBW * 3, R, 1], f32, tag="lr")
                    if USE_LRELU_ACT:
                        nc.scalar.activation(
                            lr_sb[:], t_sb[:],
                            mybir.ActivationFunctionType.Lrelu,
                            alpha=NEG_SLOPE)
                    else:
                        t2_sb = sp.tile([128, NBW * 3, R, 1], f32, tag="t2")
                        nc.vector.tensor_scalar_mul(t2_sb[:], t_sb[:],
                                                    NEG_SLOPE)
                        nc.vector.tensor_tensor(
                            out=lr_sb[:], in0=t_sb[:], in1=t2_sb[:],
                            op=mybir.AluOpType.max)
                    w_sb2 = sp.tile([128, NBW * 3, R, 1], f32, tag="w")
                    nc.scalar.activation(
                        w_sb2[:], lr_sb[:], mybir.ActivationFunctionType.Exp)
                    nc.vector.tensor_tensor(
                        out=ghr_q[:, :, :, 0:D], in0=ghr_q[:, :, :, 0:D],
                        in1=w_sb2[:].broadcast_to([128, NBW * 3, R, D]),
                        op=mybir.AluOpType.mult)
                    nc.vector.tensor_copy(ghr_q[:, :, :, D:D + 1], w_sb2[:])
                    ghr_r = ghr[:, :, 0:RW].rearrange(
                        "p (q j) e -> p q e j", j=R)
                    nc.vector.tensor_reduce(
                        out=red[:, b * NBW * 3:(b + 1) * NBW * 3, :],
                        in_=ghr_r, axis=mybir.AxisListType.X,
                        op=mybir.AluOpType.add)
                    if INTERLEAVE_MM and not ABLATE_MM:
                        for w in range(b * NBW, (b + 1) * NBW):
                            pu = mp.tile([128, UW], f32, tag="pu")
                            for k in range(3):
                                nc.tensor.matmul(
                                    pu[:], lhsT=sel[:, 3 * w + k, :],
                                    rhs=red[:, 3 * w + k, 0:UW],
                                    start=(k == 0), stop=(k == 2))
                            nc.vector.tensor_copy(ustage[:, w, :], pu[:])

            if DEBUG_RED:
                nc.sync.dma_start(red_dump[:], red[:])

            # ---------------- Phase 3: Sel matmul + epilogue --------------
            with tc.tile_pool(name="mm2", bufs=4, space="PSUM") as mp2, \
                 tc.tile_pool(name="ot", bufs=1) as otp:
                if ABLATE_MM:
                    nc.vector.memset(ustage[:], 0.25)
                elif not INTERLEAVE_MM:
                    for w in range(NW):
                        pu = mp2.tile([128, UW], f32, tag="pu")
                        for k in range(3):
                            nc.tensor.matmul(
                                pu[:], lhsT=sel[:, 3 * w + k, :],
                                rhs=red[:, 3 * w + k, 0:UW],
                                start=(k == 0), stop=(k == 2))
                        nc.vector.tensor_copy(ustage[:, w, :], pu[:])
                rcp = otp.tile([128, NW, 1], f32, tag="rcp")
                nc.vector.reciprocal(rcp[:], ustage[:, :, D:D + 1])
                ot = otp.tile([128, NW, D], f32, tag="ot")
                nc.vector.tensor_tensor(
                    out=ot[:], in0=ustage[:, :, 0:D],
                    in1=rcp[:].broadcast_to([128, NW, D]),
                    op=mybir.AluOpType.mult)
                nc.sync.dma_start(
                    out_ext.rearrange("(w p) d -> p w d", p=128), ot[:])

    nc.compile()
    return nc


def _get_program(ncores):
    if ncores not in _prog_cache:
        _prog_cache[ncores] = _build_program(ncores)
    return _prog_cache[ncores]


def kernel(feats, W, attn_l, attn_r, bias, src, dst):
    from concourse.bass_utils import run_bass_kernel_spmd

    feats = np.asarray(feats, np.float32)
    W = np.asarray(W, np.float32)
    attn_l = np.asarray(attn_l, np.float32)
    attn_r = np.asarray(attn_r, np.float32)
    bias = np.asarray(bias, np.float32)
    src = np.asarray(src)
    dst = np.asarray(dst)

    in_maps, newid = _prep_inputs(feats, W, attn_l, attn_r, bias, src, dst)
    nc = _get_program(NCORES)
    res = run_bass_kernel_spmd(nc, in_maps, list(range(NCORES)))
    shards = [np.asarray(res.results[c]["out"]) for c in range(NCORES)]
    out_cat = np.concatenate(shards, axis=0)
    return out_cat[newid[:N]].astype(np.float32)
